# revision 15
# baseline (speedup 1.0000x reference)
"""Trainium2 Bass kernel for nn_Mnist_lmdSplineKAN.

Sharding: data-parallel over batch, 8 cores x 128 rows. All params replicated.

Per-core math (I=784, H=10, O=64, 8 cubic B-spline basis fns on 5 intervals):
  ti = round(5x-0.5); u = 5x - ti; masks m_t = (ti == t)
  pieces PR[s]: PR0=u^3, PR1=3w^3-6w^2+4, PR2=3u^3-6u^2+4, PR3=w^3 (w=1-u)
  basis f_j = sum_t m_t * PR[3-j+t]; masks disjoint => each masked product
  is a valid standalone PE feature.  11 feature tiles are fed to the PE:
    f0=(t0,s3)->j0  f1=(t0,s2)->j1  f2=(t1,s3)->j1   [single products]
    f3=j2 f4=j3 f5=j4 f6=j5                          [DVE-fused sums]
    f7=(t3,s0)->j6  f8=(t4,s1)->j6  f9=(t4,s0)->j7   [single products]
    f10=silu                                         -> j8
  Each contracts against the 9-basis weight tile wg[j] (4.4 MB fp8 total).
  Products come from 5 batched broadcast TTs (GS[t] = m_t * PR[0:4]).

  fp8e4 everywhere (IEEE e4m3, max 240); per-(h,o)-column weight scales,
  descaled on the PSUM->SBUF copy.  Main matmuls: DoubleRow fp8 over chunk
  pairs, features stationary, one explicit LDWEIGHTS shared by the two
  output-half matmuls.  I = 6 chunks of 128 (3 pairs) + 16-row tail; tail
  rows of all 11 features are gathered into 2 tiles and hit with plain
  fp8 matmuls.

  Tail: ysb = y*colscale (DVE), 5x f32 transposes (PE), h1T = tanh (ACT),
  layer2 = 5 matmuls + ones-row for b1, h2 = tanh, logits = rowsum(h2*W2)
  + b2 (DVE), out (128,10) f32.

DMA: x on sync HW queue first; weights on gpsimd SWDGE (9 j-pieces in
first-use order); consts + tail weights on scalar; gathers + out on sync.
"""
import sys, types
import numpy as np

B, I, O, H, NB = 1024, 784, 64, 10, 8
NC = 8
BC = B // NC      # 128
CH = 7
PAIRS = 3
PLAST = 16
HO = H * O        # 640
HS = 5
D2 = H * 32       # 320
NF = NB + 1       # 9 weight features
FREE = CH * BC    # 896
NFEAT = 11

# feature table: (kind, payload)
#  ('p', (t, s))   single product slot -> j = t+3-s
#  ('f', j)        fused basis j (DVE adds)
#  ('s', None)     silu
FEATS = [('p', (0, 3)), ('p', (0, 2)), ('p', (1, 3)),
         ('f', 2), ('f', 3), ('f', 4), ('f', 5),
         ('p', (3, 0)), ('p', (4, 1)), ('p', (4, 0)), ('s', None)]


def _feat_j(f):
    kind, pl = FEATS[f]
    if kind == 'p':
        t, s = pl
        return t + 3 - s
    if kind == 'f':
        return pl
    return 8


# weight DMA order = first PE use order
JORDER_W = (0, 1, 2, 3, 4, 5, 6, 7, 8)


def _install_ntff_hook():
    if "antenv.axon_hooks" in sys.modules:
        return
    try:
        import antenv
        mod = types.ModuleType("antenv.axon_hooks")
        _h = [None]
        mod.set_axon_ntff_profile_hook = lambda h: _h.__setitem__(0, h)
        mod.get_axon_ntff_profile_hook = lambda: _h[0]
        sys.modules["antenv.axon_hooks"] = mod
        antenv.axon_hooks = mod
        from trn_agent_boot.trn_boot import _ntff_profile_via_ctypes
        h = _ntff_profile_via_ctypes("/opt/axon/libaxon_pjrt.so")
        if h is not None:
            mod.set_axon_ntff_profile_hook(h)
    except Exception:
        pass


_CACHE = {}


def _build():
    if "nc" in _CACHE:
        return _CACHE["nc"]
    import concourse.bacc as bacc
    import concourse.bass as bass
    import concourse.tile as tile
    from concourse import mybir
    from contextlib import ExitStack

    f32, f16, f8 = mybir.dt.float32, mybir.dt.float16, mybir.dt.float8e4
    i32 = mybir.dt.int32
    ALU = mybir.AluOpType
    AF = mybir.ActivationFunctionType
    DR = mybir.MatmulPerfMode.DoubleRow

    nc = bacc.Bacc("TRN2", target_bir_lowering=False, debug=False)
    x_d = nc.dram_tensor("x", (128, CH, BC), f32, kind="ExternalInput").ap()
    wp_d = nc.dram_tensor("wp", (NF * 128 * PAIRS * 2 * HO,), f8,
                          kind="ExternalInput").ap()
    # tail weights: 11 slots x 16 rows = 176 rows
    wt_d = nc.dram_tensor("wt", (NFEAT * PLAST, HO), f8,
                          kind="ExternalInput").ap()
    c16_d = nc.dram_tensor("c16", (128, HS * D2), f16, kind="ExternalInput").ap()
    b1_d = nc.dram_tensor("b1", (1, D2), f16, kind="ExternalInput").ap()
    c32_d = nc.dram_tensor("c32", (128, D2 + H + HO), f32,
                           kind="ExternalInput").ap()
    idt_d = nc.dram_tensor("idt", (128, 128), f32, kind="ExternalInput").ap()
    out_d = nc.dram_tensor("out", (BC, H), f32, kind="ExternalOutput").ap()

    with tile.TileContext(nc) as tc, ExitStack() as ctx:
        sb = ctx.enter_context(tc.tile_pool(name="sb", bufs=1))
        ps = ctx.enter_context(tc.tile_pool(name="ps", bufs=1, space="PSUM"))

        # ---- x split: chunks 0:4 on sync, 4:7 on gpsimd (before weights);
        #      scalar queue stays empty so ACT starts silu immediately ----
        xt = sb.tile([128, CH, BC], f32, tag="xt")
        nc.sync.dma_start(xt[:, 0:4, :], x_d[:, 0:4, :])
        nc.gpsimd.dma_start(xt[:, 4:CH, :], x_d[:, 4:CH, :])

        # ---- weights on gpsimd SWDGE ----
        wg = {}
        PIECE = 128 * PAIRS * 2 * HO
        for j in JORDER_W:
            t = sb.tile([128, PAIRS, 2, HO], f8, tag=f"wg{j}", name=f"wg{j}")
            src = bass.AP(tensor=wp_d.tensor, offset=j * PIECE,
                          ap=[[PAIRS * 2 * HO, 128], [1, PAIRS * 2 * HO]])
            nc.gpsimd.dma_start(t[:], src)
            wg[j] = t

        # ---- consts on the sync HW queue (after x) ----
        c16 = sb.tile([128, HS * D2], f16, tag="c16")
        nc.sync.dma_start(c16[:], c16_d)
        w1t = c16[:].rearrange("p (k d) -> p k d", d=D2)
        b1r = sb.tile([1, D2], f16, tag="b1r")
        nc.sync.dma_start(b1r[:], b1_d)
        c32 = sb.tile([128, D2 + H + HO], f32, tag="c32")
        nc.sync.dma_start(c32[:], c32_d)
        w2b = c32[:, 0:D2]
        b2b = c32[:, D2:D2 + H]
        sbc = c32[:, D2 + H:]
        idt = sb.tile([128, 128], f32, tag="idt")
        nc.sync.dma_start(idt[:], idt_d)
        wt0 = sb.tile([128, HO], f8, tag="wt0")
        nc.sync.dma_start(wt0[:], wt_d[0:128, :])
        wt1 = sb.tile([NFEAT * PLAST - 128, HO], f8, tag="wt1")
        nc.sync.dma_start(wt1[:], wt_d[128:NFEAT * PLAST, :])

        ones = sb.tile([1, 128], f16, tag="ones")
        nc.vector.memset(ones[:], 1.0)

        xr = xt[:].rearrange("p c b -> p (c b)")

        def T(tag, dt=f16):
            return sb.tile([128, FREE], dt, tag=tag, name=tag)

        # ---- index math on DVE ----
        ti32 = T("ti32", i32)
        nc.vector.tensor_scalar(ti32[:], xr, 5.0, -0.5, op0=ALU.mult,
                                op1=ALU.add)
        u = T("u")
        nc.vector.scalar_tensor_tensor(u[:], xr, 5.0, ti32[:], op0=ALU.mult,
                                       op1=ALU.subtract)
        # ACT kicks off u2/w2 as soon as u is ready (silu comes later)
        u2 = T("u2")
        nc.scalar.activation(u2[:], u[:], AF.Square)
        w2 = T("w2")
        nc.scalar.activation(w2[:], u[:], AF.Square, bias=1.0, scale=-1.0)
        # DVE hides the ACT round-trip behind masks
        tif = T("tif")
        nc.vector.tensor_copy(tif[:], ti32[:])
        M = sb.tile([128, 5, FREE], f16, tag="M")
        for t in range(5):
            nc.vector.tensor_scalar(M[:, t, :], tif[:], float(t), None,
                                    op0=ALU.is_equal)
        w_ = T("w_")
        nc.vector.tensor_scalar(w_[:], u[:], -1.0, 1.0, op0=ALU.mult,
                                op1=ALU.add)
        PR = sb.tile([128, 4, FREE], f16, tag="PR")
        nc.vector.tensor_tensor(PR[:, 0, :], u[:], u2[:], op=ALU.mult)   # u^3
        nc.vector.tensor_tensor(PR[:, 3, :], w_[:], w2[:], op=ALU.mult)  # w^3
        rw = T("rw"); rw2 = T("rw2")
        nc.vector.tensor_scalar(rw[:], w2[:], -2.0, None, op0=ALU.mult)
        nc.vector.tensor_tensor(rw2[:], rw[:], PR[:, 3, :], op=ALU.add)
        nc.vector.tensor_scalar(PR[:, 1, :], rw2[:], 3.0, 4.0, op0=ALU.mult,
                                op1=ALU.add)                     # 3w^3-6w^2+4
        ru = T("ru"); ru2 = T("ru2")
        nc.vector.tensor_scalar(ru[:], u2[:], -2.0, None, op0=ALU.mult)
        nc.vector.tensor_tensor(ru2[:], ru[:], PR[:, 0, :], op=ALU.add)
        nc.vector.tensor_scalar(PR[:, 2, :], ru2[:], 3.0, 4.0, op0=ALU.mult,
                                op1=ALU.add)                     # 3u^3-6u^2+4

        # ---- products GS[t] = m_t (bcast) * PR[0:4]; fused adds for j=2..5
        GS = {}
        for t in range(5):
            GS[t] = sb.tile([128, 4, FREE], f16, tag=f"GS{t}", name=f"GS{t}")
        FS = {j: T(f"FS{j}") for j in (2, 3, 4, 5)}
        aa = T("aa"); bb = T("bb"); aa2 = T("aa2"); bb2 = T("bb2")

        def gs_prod(t):
            mslice = M[:, t, :]
            mb = bass.AP(tensor=mslice.tensor, offset=mslice.offset,
                         ap=[list(mslice.ap[0]), [0, 4], [1, FREE]])
            nc.vector.tensor_tensor(GS[t][:], mb, PR[:], op=ALU.mult)

        # slot of basis j within GS[t]: s = t+3-j
        def gsl(t, j):
            return GS[t][:, t + 3 - j, :]

        gs_prod(0)
        gs_prod(1)
        gs_prod(2)
        # j2 = t0s1 + t1s2 + t2s3
        nc.vector.tensor_tensor(aa[:], gsl(0, 2), gsl(1, 2), op=ALU.add)
        nc.vector.tensor_tensor(FS[2][:], aa[:], gsl(2, 2), op=ALU.add)
        gs_prod(3)
        # j3 = t0s0 + t1s1 + t2s2 + t3s3
        nc.vector.tensor_tensor(bb[:], gsl(0, 3), gsl(1, 3), op=ALU.add)
        nc.vector.tensor_tensor(aa2[:], gsl(2, 3), gsl(3, 3), op=ALU.add)
        nc.vector.tensor_tensor(FS[3][:], bb[:], aa2[:], op=ALU.add)
        # GS4 in two halves: slots s2,s3 first (feed j5/j4 fused sums),
        # then s0,s1 (single-product features f8/f9)
        m4s = M[:, 4, :]
        m4b = bass.AP(tensor=m4s.tensor, offset=m4s.offset,
                      ap=[list(m4s.ap[0]), [0, 2], [1, FREE]])
        nc.vector.tensor_tensor(GS[4][:, 2:4, :], m4b, PR[:, 2:4, :],
                                op=ALU.mult)
        # j5 = t2s0 + t3s1 + t4s2
        nc.vector.tensor_tensor(bb[:], gsl(2, 5), gsl(3, 5), op=ALU.add)
        nc.vector.tensor_tensor(FS[5][:], bb[:], gsl(4, 5), op=ALU.add)
        # j4 = t1s0 + t2s1 + t3s2 + t4s3
        nc.vector.tensor_tensor(bb2[:], gsl(1, 4), gsl(2, 4), op=ALU.add)
        nc.vector.tensor_tensor(aa[:], gsl(3, 4), gsl(4, 4), op=ALU.add)
        nc.vector.tensor_tensor(FS[4][:], bb2[:], aa[:], op=ALU.add)
        nc.vector.tensor_tensor(GS[4][:, 0:2, :], m4b, PR[:, 0:2, :],
                                op=ALU.mult)

        # ---- fp8 feature tiles + converts on ACT (completion order) ----
        fq = {}
        for f in range(NFEAT):
            fq[f] = sb.tile([128, CH, BC], f8, tag=f"fq{f}", name=f"fq{f}")

        def conv(f):
            kind, pl = FEATS[f]
            if kind == 'p':
                t, s = pl
                src = GS[t][:, s, :]
            elif kind == 'f':
                src = FS[pl][:]
            else:
                nc.scalar.activation(
                    fq[f][:].rearrange("p c b -> p (c b)"), xr, AF.Silu)
                return
            nc.scalar.activation(fq[f][:].rearrange("p c b -> p (c b)"),
                                 src, AF.Copy)

        # order converts by source availability: GS0 slots, GS1, j2, GS3…
        CONV_ORDER = [0, 1, 2, 3, 7, 4, 8, 9, 5, 6, 10]
        for f in CONV_ORDER:
            conv(f)

        # ---- main matmuls: DoubleRow fp8, features stationary ----
        psum = [ps.tile([128, D2], f32, tag=f"y{nh}", name=f"y{nh}")
                for nh in range(2)]
        for f in range(NFEAT):
            j = _feat_j(f)
            for p in range(PAIRS):
                for nh in range(2):
                    nc.tensor.matmul(
                        psum[nh][:],
                        fq[f][:, 2 * p:2 * p + 2, :],
                        wg[j][:, p, :, nh * D2:(nh + 1) * D2],
                        start=(f == 0 and p == 0), stop=False,
                        perf_mode=DR)

        # ---- tail: gather 16-row slices of all features, 2 plain matmuls
        gath0 = sb.tile([128, BC], f8, tag="ga0")
        gath1 = sb.tile([48, BC], f8, tag="ga1")
        for f in range(8):
            nc.sync.dma_start(gath0[16 * f:16 * f + 16, :],
                              fq[f][0:PLAST, 6, :])
        for f in range(8, NFEAT):
            nc.sync.dma_start(gath1[16 * (f - 8):16 * (f - 8) + 16, :],
                              fq[f][0:PLAST, 6, :])
        for nh in range(2):
            nc.tensor.matmul(psum[nh][:], gath0[:],
                             wt0[:, nh * D2:(nh + 1) * D2],
                             start=False, stop=False)
            nc.tensor.matmul(psum[nh][:], gath1[:],
                             wt1[:, nh * D2:(nh + 1) * D2],
                             start=False, stop=True)

        # ---- tail: descale, transpose, tanh, layer2, reduce ----
        ysb = sb.tile([128, HO], f32, tag="ysb")
        for nh in range(2):
            nc.vector.tensor_tensor(ysb[:, nh * D2:(nh + 1) * D2],
                                    psum[nh][:],
                                    sbc[:, nh * D2:(nh + 1) * D2],
                                    op=ALU.mult)
        h1t = []
        for k in range(HS):
            pt = ps.tile([128, 128], f32, tag=f"pt{k}", name=f"pt{k}")
            nc.tensor.transpose(pt[:], ysb[:, k * 128:(k + 1) * 128], idt[:])
            st = sb.tile([128, 128], f16, tag=f"h1t{k}", name=f"h1t{k}")
            nc.scalar.activation(st[:], pt[:], AF.Tanh)
            h1t.append(st)
        ps2 = ps.tile([128, D2], f32, tag="ps2")
        for k in range(HS):
            nc.tensor.matmul(ps2[:], h1t[k][:], w1t[:, k, :],
                             start=(k == 0), stop=False)
        nc.tensor.matmul(ps2[:], ones[:], b1r[:], start=False, stop=True)
        h2 = sb.tile([128, D2], f32, tag="h2")
        nc.scalar.activation(h2[:], ps2[:], AF.Tanh)
        prod = sb.tile([128, D2], f32, tag="prod")
        nc.vector.tensor_tensor(prod[:], h2[:], w2b, op=ALU.mult)
        red = sb.tile([128, H], f32, tag="red")
        nc.vector.tensor_reduce(red[:], prod[:].rearrange("p (h d) -> p h d",
                                                          d=32),
                                axis=mybir.AxisListType.X, op=ALU.add)
        lg = sb.tile([128, H], f32, tag="lg")
        nc.vector.tensor_tensor(lg[:], red[:], b2b, op=ALU.add)
        nc.sync.dma_start(out_d, lg[:])

    nc.compile()
    _CACHE["nc"] = nc
    return nc


def _prep_inputs(x, coef, scale_base, scale_sp, lmd, W1, b1, W2, b2):
    import ml_dtypes
    E4 = ml_dtypes.float8_e4m3   # TRN2 fp8e4: IEEE e4m3, max normal 240
    xf = np.asarray(x, np.float64).reshape(B, I)
    coef = np.asarray(coef, np.float64)
    eff = coef * np.asarray(scale_sp, np.float64)[..., None] \
        * np.asarray(lmd, np.float64)[:, :, None, None] / 6.0
    sbl = np.asarray(scale_base, np.float64) \
        * np.asarray(lmd, np.float64)[:, :, None]
    wbig = np.concatenate([eff, sbl[..., None]], -1)           # (H,I,O,9)
    wi = np.ascontiguousarray(wbig.transpose(1, 3, 0, 2)).reshape(I, NF, HO)
    s_col = np.abs(wi).max(axis=(0, 1)) / 240.0 * 1.05         # (640,)
    s_col[s_col == 0] = 1.0
    wq = np.asarray(wi / s_col[None, None, :], E4)             # (I, NF, HO)

    wq8 = wq.view(np.uint8)
    wp = np.empty((NF, 128, PAIRS, 2, HO), np.uint8)
    for p in range(PAIRS):
        for k in range(2):
            blk = wq8[p * 256 + k * 128: p * 256 + (k + 1) * 128]
            wp[:, :, p, k, :] = blk.transpose(1, 0, 2)
    wp = wp.reshape(-1).view(E4)
    # tail rows: feature-slot-major, 16 tail I-rows each
    wt = np.empty((NFEAT * PLAST, HO), np.uint8)
    for f in range(NFEAT):
        j = _feat_j(f)
        wt[f * PLAST:(f + 1) * PLAST, :] = wq8[768:I, j, :]
    wt = wt.view(E4)

    W1 = np.asarray(W1, np.float64)
    w1bd = np.zeros((HO, D2))
    for h in range(H):
        w1bd[h * O:(h + 1) * O, h * 32:(h + 1) * 32] = W1[h]
    c16 = np.ascontiguousarray(
        w1bd.reshape(HS, 128, D2).transpose(1, 0, 2)).astype(
            np.float16).reshape(128, HS * D2)
    b1c = np.asarray(b1, np.float16).reshape(1, D2).copy()
    c32 = np.ascontiguousarray(np.concatenate([
        np.broadcast_to(np.asarray(W2, np.float32).reshape(D2), (128, D2)),
        np.broadcast_to(np.asarray(b2, np.float32).reshape(H), (128, H)),
        np.broadcast_to(s_col.astype(np.float32), (128, HO))],
        1).astype(np.float32))
    idt = np.eye(128, dtype=np.float32)

    in_maps = []
    for core in range(NC):
        xs = xf[core * BC:(core + 1) * BC].T
        xdev = np.zeros((128, CH, BC), np.float32)
        for c in range(CH):
            rows = xs[c * 128:min((c + 1) * 128, I)]
            xdev[0:rows.shape[0], c, :] = rows
        in_maps.append({"x": xdev, "wp": wp, "wt": wt, "c16": c16,
                        "b1": b1c, "c32": c32, "idt": idt})
    return in_maps


def run(inputs, trace=False, tmpdir=None):
    _install_ntff_hook()
    from concourse.bass_utils import run_bass_kernel_spmd
    nc = _build()
    in_maps = _prep_inputs(**inputs)
    res = run_bass_kernel_spmd(nc, in_maps, core_ids=list(range(NC)),
                               trace=trace, tmpdir=tmpdir)
    out = np.concatenate([r["out"] for r in res.results], 0)
    return out.astype(np.float32), res


def kernel(**inputs):
    out, _ = run(inputs)
    return out


# revision 20
# speedup vs baseline: 1.0326x; 1.0326x over previous
"""Trainium2 Bass kernel for nn_Mnist_lmdSplineKAN.

Sharding: data-parallel over batch, 8 cores x 128 rows. All params replicated.

Per-core math (I=784, H=10, O=64, 8 cubic B-spline basis fns on 5 intervals):
  ti = round(5x-0.5); u = 5x - ti; masks m_t = (ti == t)
  pieces PR[s]: PR0=u^3, PR1=3w^3-6w^2+4, PR2=3u^3-6u^2+4, PR3=w^3 (w=1-u)
  basis f_j = sum_t m_t * PR[3-j+t]; masks disjoint => each masked product
  is a valid standalone PE feature.  11 feature tiles are fed to the PE:
    f0=(t0,s3)->j0  f1=(t0,s2)->j1  f2=(t1,s3)->j1   [single products]
    f3=j2 f4=j3 f5=j4 f6=j5                          [DVE-fused sums]
    f7=(t3,s0)->j6  f8=(t4,s1)->j6  f9=(t4,s0)->j7   [single products]
    f10=silu                                         -> j8
  Each contracts against the 9-basis weight tile wg[j] (4.4 MB fp8 total).
  Products come from 5 batched broadcast TTs (GS[t] = m_t * PR[0:4]).

  fp8e4 everywhere (IEEE e4m3, max 240); per-(h,o)-column weight scales,
  descaled on the PSUM->SBUF copy.  Main matmuls: DoubleRow fp8 over chunk
  pairs, features stationary, one explicit LDWEIGHTS shared by the two
  output-half matmuls.  I = 6 chunks of 128 (3 pairs) + 16-row tail; tail
  rows of all 11 features are gathered into 2 tiles and hit with plain
  fp8 matmuls.

  Tail: ysb = y*colscale (DVE), 5x f32 transposes (PE), h1T = tanh (ACT),
  layer2 = 5 matmuls + ones-row for b1, h2 = tanh, logits = rowsum(h2*W2)
  + b2 (DVE), out (128,10) f32.

DMA: x on sync HW queue first; weights on gpsimd SWDGE (9 j-pieces in
first-use order); consts + tail weights on scalar; gathers + out on sync.
"""
import sys, types
import numpy as np

B, I, O, H, NB = 1024, 784, 64, 10, 8
NC = 8
BC = B // NC      # 128
CH = 7
PAIRS = 3
PLAST = 16
HO = H * O        # 640
HS = 5
D2 = H * 32       # 320
NF = NB + 1       # 9 weight features
FREE = CH * BC    # 896
NFEAT = 9

# feature table: (kind, payload)
#  ('p', (t, s))   single product slot -> j = t+3-s
#  ('f', j)        fused basis j (DVE adds)
#  ('s', None)     silu
FEATS = [('p', (0, 3)), ('f', 1), ('f', 2), ('f', 3), ('f', 4), ('f', 5),
         ('f', 6), ('p', (4, 0)), ('s', None)]


def _feat_j(f):
    kind, pl = FEATS[f]
    if kind == 'p':
        t, s = pl
        return t + 3 - s
    if kind == 'f':
        return pl
    return 8


# weight DMA order = first PE use order
JORDER_W = (0, 1, 2, 3, 4, 5, 6, 7, 8)


def _install_ntff_hook():
    if "antenv.axon_hooks" in sys.modules:
        return
    try:
        import antenv
        mod = types.ModuleType("antenv.axon_hooks")
        _h = [None]
        mod.set_axon_ntff_profile_hook = lambda h: _h.__setitem__(0, h)
        mod.get_axon_ntff_profile_hook = lambda: _h[0]
        sys.modules["antenv.axon_hooks"] = mod
        antenv.axon_hooks = mod
        from trn_agent_boot.trn_boot import _ntff_profile_via_ctypes
        h = _ntff_profile_via_ctypes("/opt/axon/libaxon_pjrt.so")
        if h is not None:
            mod.set_axon_ntff_profile_hook(h)
    except Exception:
        pass


_CACHE = {}


def _build():
    if "nc" in _CACHE:
        return _CACHE["nc"]
    import concourse.bacc as bacc
    import concourse.bass as bass
    import concourse.tile as tile
    from concourse import mybir
    from contextlib import ExitStack

    f32, f16, f8 = mybir.dt.float32, mybir.dt.float16, mybir.dt.float8e4
    i32 = mybir.dt.int32
    ALU = mybir.AluOpType
    AF = mybir.ActivationFunctionType
    DR = mybir.MatmulPerfMode.DoubleRow

    nc = bacc.Bacc("TRN2", target_bir_lowering=False, debug=False)
    x_d = nc.dram_tensor("x", (128, CH, BC), f32, kind="ExternalInput").ap()
    wp_d = nc.dram_tensor("wp", (NF * 128 * PAIRS * 2 * HO,), f8,
                          kind="ExternalInput").ap()
    # tail weights: 11 slots x 16 rows = 176 rows
    wt_d = nc.dram_tensor("wt", (NFEAT * PLAST, HO), f8,
                          kind="ExternalInput").ap()
    c16_d = nc.dram_tensor("c16", (128, HS * D2), f16, kind="ExternalInput").ap()
    b1_d = nc.dram_tensor("b1", (1, D2), f16, kind="ExternalInput").ap()
    c32_d = nc.dram_tensor("c32", (128, D2 + H + HO), f32,
                           kind="ExternalInput").ap()
    idt_d = nc.dram_tensor("idt", (128, 128), f32, kind="ExternalInput").ap()
    out_d = nc.dram_tensor("out", (BC, H), f32, kind="ExternalOutput").ap()

    with tile.TileContext(nc) as tc, ExitStack() as ctx:
        sb = ctx.enter_context(tc.tile_pool(name="sb", bufs=1))
        ps = ctx.enter_context(tc.tile_pool(name="ps", bufs=1, space="PSUM"))

        # ---- x split: chunks 0:4 on sync, 4:7 on gpsimd (before weights);
        #      scalar queue stays empty so ACT starts silu immediately ----
        xt = sb.tile([128, CH, BC], f32, tag="xt")
        nc.sync.dma_start(xt[:, 0:4, :], x_d[:, 0:4, :])
        nc.scalar.dma_start(xt[:, 4:CH, :], x_d[:, 4:CH, :])

        # ---- weights on gpsimd SWDGE ----
        wg = {}
        PIECE = 128 * PAIRS * 2 * HO
        for j in JORDER_W:
            t = sb.tile([128, PAIRS, 2, HO], f8, tag=f"wg{j}", name=f"wg{j}")
            src = bass.AP(tensor=wp_d.tensor, offset=j * PIECE,
                          ap=[[PAIRS * 2 * HO, 128], [1, PAIRS * 2 * HO]])
            nc.gpsimd.dma_start(t[:], src)
            wg[j] = t

        # ---- consts on the sync HW queue (after x) ----
        c16 = sb.tile([128, HS * D2], f16, tag="c16")
        nc.sync.dma_start(c16[:], c16_d)
        w1t = c16[:].rearrange("p (k d) -> p k d", d=D2)
        b1r = sb.tile([1, D2], f16, tag="b1r")
        nc.sync.dma_start(b1r[:], b1_d)
        c32 = sb.tile([128, D2 + H + HO], f32, tag="c32")
        nc.sync.dma_start(c32[:], c32_d)
        w2b = c32[:, 0:D2]
        b2b = c32[:, D2:D2 + H]
        sbc = c32[:, D2 + H:]
        idt = sb.tile([128, 128], f32, tag="idt")
        nc.sync.dma_start(idt[:], idt_d)
        wt0 = sb.tile([128, HO], f8, tag="wt0")
        nc.sync.dma_start(wt0[:], wt_d[0:128, :])
        wt1 = sb.tile([NFEAT * PLAST - 128, HO], f8, tag="wt1")
        nc.sync.dma_start(wt1[:], wt_d[128:NFEAT * PLAST, :])

        ones = sb.tile([1, 128], f16, tag="ones")
        nc.vector.memset(ones[:], 1.0)

        xr = xt[:].rearrange("p c b -> p (c b)")

        def T(tag, dt=f16):
            return sb.tile([128, FREE], dt, tag=tag, name=tag)

        # ---- index math on DVE ----
        ti32 = T("ti32", i32)
        nc.vector.tensor_scalar(ti32[:], xr, 5.0, -0.5, op0=ALU.mult,
                                op1=ALU.add)
        u = T("u")
        nc.vector.scalar_tensor_tensor(u[:], xr, 5.0, ti32[:], op0=ALU.mult,
                                       op1=ALU.subtract)
        # ACT kicks off u2/w2 as soon as u is ready (silu comes later)
        u2 = T("u2")
        nc.scalar.activation(u2[:], u[:], AF.Square)
        w2 = T("w2")
        nc.scalar.activation(w2[:], u[:], AF.Square, bias=1.0, scale=-1.0)
        # DVE hides the ACT round-trip behind masks
        tif = T("tif")
        nc.vector.tensor_copy(tif[:], ti32[:])
        M = sb.tile([128, 5, FREE], f16, tag="M")
        for t in range(5):
            nc.vector.tensor_scalar(M[:, t, :], tif[:], float(t), None,
                                    op0=ALU.is_equal)
        w_ = T("w_")
        nc.vector.tensor_scalar(w_[:], u[:], -1.0, 1.0, op0=ALU.mult,
                                op1=ALU.add)
        PR = sb.tile([128, 4, FREE], f16, tag="PR")
        nc.vector.tensor_tensor(PR[:, 0, :], u[:], u2[:], op=ALU.mult)   # u^3
        nc.vector.tensor_tensor(PR[:, 3, :], w_[:], w2[:], op=ALU.mult)  # w^3
        rw = T("rw"); rw2 = T("rw2")
        nc.vector.tensor_scalar(rw[:], w2[:], -2.0, None, op0=ALU.mult)
        nc.vector.tensor_tensor(rw2[:], rw[:], PR[:, 3, :], op=ALU.add)
        nc.vector.tensor_scalar(PR[:, 1, :], rw2[:], 3.0, 4.0, op0=ALU.mult,
                                op1=ALU.add)                     # 3w^3-6w^2+4
        ru = T("ru"); ru2 = T("ru2")
        nc.vector.tensor_scalar(ru[:], u2[:], -2.0, None, op0=ALU.mult)
        nc.vector.tensor_tensor(ru2[:], ru[:], PR[:, 0, :], op=ALU.add)
        nc.vector.tensor_scalar(PR[:, 2, :], ru2[:], 3.0, 4.0, op0=ALU.mult,
                                op1=ALU.add)                     # 3u^3-6u^2+4

        # ---- products GS[t] = m_t (bcast) * PR; fused adds for j=1..6 ----
        GS = {}
        for t in range(5):
            GS[t] = sb.tile([128, 4, FREE], f16, tag=f"GS{t}", name=f"GS{t}")
        FS = {j: T(f"FS{j}") for j in (1, 2, 3, 5, 6)}
        aa = T("aa"); bb = T("bb"); aa2 = T("aa2"); bb2 = T("bb2")

        fq = {}
        for f in range(NFEAT):
            fq[f] = sb.tile([128, CH, BC], f8, tag=f"fq{f}", name=f"fq{f}")

        def gs_prod(t):
            mslice = M[:, t, :]
            mb = bass.AP(tensor=mslice.tensor, offset=mslice.offset,
                         ap=[list(mslice.ap[0]), [0, 4], [1, FREE]])
            nc.vector.tensor_tensor(GS[t][:], mb, PR[:], op=ALU.mult)

        # slot of basis j within GS[t]: s = t+3-j
        def gsl(t, j):
            return GS[t][:, t + 3 - j, :]

        gs_prod(0)
        gs_prod(1)
        # j1 = t0s2 + t1s3
        nc.vector.tensor_tensor(FS[1][:], gsl(0, 1), gsl(1, 1), op=ALU.add)
        gs_prod(2)
        # j2 = t0s1 + t1s2 + t2s3
        nc.vector.tensor_tensor(aa[:], gsl(0, 2), gsl(1, 2), op=ALU.add)
        nc.vector.tensor_tensor(FS[2][:], aa[:], gsl(2, 2), op=ALU.add)
        gs_prod(3)
        # j3 = t0s0 + t1s1 + t2s2 + t3s3
        nc.vector.tensor_tensor(bb[:], gsl(0, 3), gsl(1, 3), op=ALU.add)
        nc.vector.tensor_tensor(aa2[:], gsl(2, 3), gsl(3, 3), op=ALU.add)
        nc.vector.tensor_tensor(FS[3][:], bb[:], aa2[:], op=ALU.add)
        gs_prod(4)
        # j5 = t2s0 + t3s1 + t4s2
        nc.vector.tensor_tensor(bb[:], gsl(2, 5), gsl(3, 5), op=ALU.add)
        nc.vector.tensor_tensor(FS[5][:], bb[:], gsl(4, 5), op=ALU.add)
        # j6 = t3s0 + t4s1
        nc.vector.tensor_tensor(FS[6][:], gsl(3, 6), gsl(4, 6), op=ALU.add)
        # j4 = t1s0 + t2s1 + t3s2 + t4s3 -- final add straight to fp8
        nc.vector.tensor_tensor(bb2[:], gsl(1, 4), gsl(2, 4), op=ALU.add)
        nc.vector.tensor_tensor(aa[:], gsl(3, 4), gsl(4, 4), op=ALU.add)
        nc.vector.tensor_tensor(fq[4][:].rearrange("p c b -> p (c b)"),
                                bb2[:], aa[:], op=ALU.add)

        # ---- fp8 converts on ACT (completion order; j4 done on DVE) ----
        def conv(f):
            kind, pl = FEATS[f]
            if kind == 'p':
                t, s = pl
                src = GS[t][:, s, :]
            elif kind == 'f':
                src = FS[pl][:]
            else:
                nc.scalar.activation(
                    fq[f][:].rearrange("p c b -> p (c b)"), xr, AF.Silu)
                return
            nc.scalar.activation(fq[f][:].rearrange("p c b -> p (c b)"),
                                 src, AF.Copy)

        for f in (8, 0, 1, 2, 3, 7, 5, 6):
            conv(f)

        # ---- main matmuls: DoubleRow fp8, features stationary ----
        psum = [ps.tile([128, D2], f32, tag=f"y{nh}", name=f"y{nh}")
                for nh in range(2)]
        for f in range(NFEAT):
            j = _feat_j(f)
            for p in range(PAIRS):
                for nh in range(2):
                    nc.tensor.matmul(
                        psum[nh][:],
                        fq[f][:, 2 * p:2 * p + 2, :],
                        wg[j][:, p, :, nh * D2:(nh + 1) * D2],
                        start=(f == 0 and p == 0), stop=False,
                        perf_mode=DR)

        # ---- tail: gather 16-row slices of all features, 2 plain matmuls
        gath0 = sb.tile([128, BC], f8, tag="ga0")
        gath1 = sb.tile([PLAST, BC], f8, tag="ga1")
        for f in range(8):
            nc.sync.dma_start(gath0[16 * f:16 * f + 16, :],
                              fq[f][0:PLAST, 6, :])
        nc.sync.dma_start(gath1[:], fq[8][0:PLAST, 6, :])
        for nh in range(2):
            nc.tensor.matmul(psum[nh][:], gath0[:],
                             wt0[:, nh * D2:(nh + 1) * D2],
                             start=False, stop=False)
            nc.tensor.matmul(psum[nh][:], gath1[:],
                             wt1[:, nh * D2:(nh + 1) * D2],
                             start=False, stop=True)

        # ---- tail: descale, transpose, tanh, layer2, reduce ----
        ysb = sb.tile([128, HO], f32, tag="ysb")
        for nh in range(2):
            nc.vector.tensor_tensor(ysb[:, nh * D2:(nh + 1) * D2],
                                    psum[nh][:],
                                    sbc[:, nh * D2:(nh + 1) * D2],
                                    op=ALU.mult)
        h1t = []
        for k in range(HS):
            pt = ps.tile([128, 128], f32, tag=f"pt{k}", name=f"pt{k}")
            nc.tensor.transpose(pt[:], ysb[:, k * 128:(k + 1) * 128], idt[:])
            st = sb.tile([128, 128], f16, tag=f"h1t{k}", name=f"h1t{k}")
            nc.scalar.activation(st[:], pt[:], AF.Tanh)
            h1t.append(st)
        ps2 = ps.tile([128, D2], f32, tag="ps2")
        for k in range(HS):
            nc.tensor.matmul(ps2[:], h1t[k][:], w1t[:, k, :],
                             start=(k == 0), stop=False)
        nc.tensor.matmul(ps2[:], ones[:], b1r[:], start=False, stop=True)
        h2 = sb.tile([128, D2], f32, tag="h2")
        nc.scalar.activation(h2[:], ps2[:], AF.Tanh)
        prod = sb.tile([128, D2], f32, tag="prod")
        nc.vector.tensor_tensor(prod[:], h2[:], w2b, op=ALU.mult)
        red = sb.tile([128, H], f32, tag="red")
        nc.vector.tensor_reduce(red[:], prod[:].rearrange("p (h d) -> p h d",
                                                          d=32),
                                axis=mybir.AxisListType.X, op=ALU.add)
        lg = sb.tile([128, H], f32, tag="lg")
        nc.vector.tensor_tensor(lg[:], red[:], b2b, op=ALU.add)
        nc.sync.dma_start(out_d, lg[:])

    nc.compile()
    _CACHE["nc"] = nc
    return nc


def _prep_inputs(x, coef, scale_base, scale_sp, lmd, W1, b1, W2, b2):
    import ml_dtypes
    E4 = ml_dtypes.float8_e4m3   # TRN2 fp8e4: IEEE e4m3, max normal 240
    xf = np.asarray(x, np.float64).reshape(B, I)
    coef = np.asarray(coef, np.float64)
    eff = coef * np.asarray(scale_sp, np.float64)[..., None] \
        * np.asarray(lmd, np.float64)[:, :, None, None] / 6.0
    sbl = np.asarray(scale_base, np.float64) \
        * np.asarray(lmd, np.float64)[:, :, None]
    wbig = np.concatenate([eff, sbl[..., None]], -1)           # (H,I,O,9)
    wi = np.ascontiguousarray(wbig.transpose(1, 3, 0, 2)).reshape(I, NF, HO)
    s_col = np.abs(wi).max(axis=(0, 1)) / 240.0 * 1.05         # (640,)
    s_col[s_col == 0] = 1.0
    wq = np.asarray(wi / s_col[None, None, :], E4)             # (I, NF, HO)

    wq8 = wq.view(np.uint8)
    wp = np.empty((NF, 128, PAIRS, 2, HO), np.uint8)
    for p in range(PAIRS):
        for k in range(2):
            blk = wq8[p * 256 + k * 128: p * 256 + (k + 1) * 128]
            wp[:, :, p, k, :] = blk.transpose(1, 0, 2)
    wp = wp.reshape(-1).view(E4)
    # tail rows: feature-slot-major, 16 tail I-rows each
    wt = np.empty((NFEAT * PLAST, HO), np.uint8)
    for f in range(NFEAT):
        j = _feat_j(f)
        wt[f * PLAST:(f + 1) * PLAST, :] = wq8[768:I, j, :]
    wt = wt.view(E4)

    W1 = np.asarray(W1, np.float64)
    w1bd = np.zeros((HO, D2))
    for h in range(H):
        w1bd[h * O:(h + 1) * O, h * 32:(h + 1) * 32] = W1[h]
    c16 = np.ascontiguousarray(
        w1bd.reshape(HS, 128, D2).transpose(1, 0, 2)).astype(
            np.float16).reshape(128, HS * D2)
    b1c = np.asarray(b1, np.float16).reshape(1, D2).copy()
    c32 = np.ascontiguousarray(np.concatenate([
        np.broadcast_to(np.asarray(W2, np.float32).reshape(D2), (128, D2)),
        np.broadcast_to(np.asarray(b2, np.float32).reshape(H), (128, H)),
        np.broadcast_to(s_col.astype(np.float32), (128, HO))],
        1).astype(np.float32))
    idt = np.eye(128, dtype=np.float32)

    in_maps = []
    for core in range(NC):
        xs = xf[core * BC:(core + 1) * BC].T
        xdev = np.zeros((128, CH, BC), np.float32)
        for c in range(CH):
            rows = xs[c * 128:min((c + 1) * 128, I)]
            xdev[0:rows.shape[0], c, :] = rows
        in_maps.append({"x": xdev, "wp": wp, "wt": wt, "c16": c16,
                        "b1": b1c, "c32": c32, "idt": idt})
    return in_maps


def run(inputs, trace=False, tmpdir=None):
    _install_ntff_hook()
    from concourse.bass_utils import run_bass_kernel_spmd
    nc = _build()
    in_maps = _prep_inputs(**inputs)
    res = run_bass_kernel_spmd(nc, in_maps, core_ids=list(range(NC)),
                               trace=trace, tmpdir=tmpdir)
    out = np.concatenate([r["out"] for r in res.results], 0)
    return out.astype(np.float32), res


def kernel(**inputs):
    out, _ = run(inputs)
    return out
